# revision 5
# baseline (speedup 1.0000x reference)
"""Distributed attention kernel for Trainium2 (8 NeuronCores).

Problem: out = softmax((x_q W_q^T)(x_k W_k^T)^T / sqrt(D)) (x_v W_v^T)
with SEQ=4096, D=1024, all f32.

Strategy (sequence parallel, sharded projections):
  - Shard all three encodings along the sequence dim: core c owns rows
    [c*512, (c+1)*512).
  - Host-side prep: transpose + cast to bf16 + permute into the exact
    SBUF tile layout [128, ...] so every DMA moves fat contiguous
    per-partition lines (4-16KB descriptors).
  - Each core computes Q^T (kept local), K^T and V for its shard, then
    AllGathers K^T and V across the 8 cores. Both gathers are split in
    two chunks (K^T along k, V along the output d-chunk) so downstream
    compute starts on the first half while the second is in flight.
  - simsT[k, q] = sum_e KT[e, k] * QT[e, q] computed tile-by-tile
    (output already transposed so attn@V needs no on-chip transpose of
    the softmax matrix).
  - exp via ScalarE with fused 1/32 scale. Logits are ~N(0,1) so the
    max-subtraction is unnecessary for f32/bf16 range.
  - attn@V in two passes over 512-wide d-chunks; the softmax denominator
    rides along as N=1 matmuls against a ones vector (same stationary
    operand, so the extra LDWEIGHTS pipelines away). Each PSUM
    accumulation chain owns a full bank (start=True zeroes the whole 2KB
    zero region), so the 4 denominator chains are split across the two
    d-passes: 4 acc banks + 2 den banks + 2 work banks = 8.
  - Row-normalize by 1/den, DMA out f32. The dc=0 results for q-subtiles
    2,3 are stashed unnormalized in SBUF until their denominators finish
    in the dc=1 pass.
"""

import numpy as np
import ml_dtypes

N_CORES = 8
SEQ = 4096
D = 1024
S_SH = SEQ // N_CORES  # 512 rows per core
P = 128
NT = D // P            # 8 tiles along d / e
NKT = SEQ // P         # 32 k-tiles
NQS = S_SH // P        # 4 q (and local-k) sub-tiles
FD = 512               # matmul free dim (one PSUM bank)
KH = S_SH // 2         # 256, k-half for the chunked K^T gather
NDC = D // FD          # 2 output d-chunks
SCALE = 1.0 / float(np.sqrt(D))

_CACHE = {}


def _body(tc, nc, mybir, xqt, xkt, xvt, wqt, wkt, wvt, out):
    bf16 = mybir.dt.bfloat16
    fp32 = mybir.dt.float32
    RG = [list(range(N_CORES))]

    def allgather(src, dst):
        nc.gpsimd.collective_compute(
            "AllGather",
            mybir.AluOpType.bypass,
            replica_groups=RG,
            ins=[src[:].opt()],
            outs=[dst[:].opt()],
        )

    def load_chunked(engine, sb, dram_t, nchunk=4):
        step = NT // nchunk
        for i in range(nchunk):
            engine.dma_start(sb[:, i * step:(i + 1) * step], dram_t[:, i * step:(i + 1) * step])

    with (
        tc.tile_pool(name="dram", bufs=1, space="DRAM") as dram,
        tc.tile_pool(name="wx", bufs=1) as wx,
        tc.tile_pool(name="persist", bufs=1) as persist,
        tc.tile_pool(name="stage", bufs=2) as stage,
        tc.tile_pool(name="stream", bufs=3) as stream,
        tc.tile_pool(name="outp", bufs=4) as outp,
        tc.tile_pool(name="psw", bufs=2, space="PSUM") as psw,
        tc.tile_pool(name="psacc", bufs=4, space="PSUM") as psacc,
        tc.tile_pool(name="psden", bufs=2, space="PSUM") as psden,
    ):
        kt_in_a = dram.tile([P, NT, KH], bf16)
        kt_in_b = dram.tile([P, NT, KH], bf16)
        ktg_a = dram.tile([N_CORES, P, NT, KH], bf16, addr_space="Shared")
        ktg_b = dram.tile([N_CORES, P, NT, KH], bf16, addr_space="Shared")
        v_in_a = dram.tile([P, NQS, FD], bf16)
        v_in_b = dram.tile([P, NQS, FD], bf16)
        vg_a = dram.tile([N_CORES, P, NQS, FD], bf16, addr_space="Shared")
        vg_b = dram.tile([N_CORES, P, NQS, FD], bf16, addr_space="Shared")

        # ---- P1a: K^T projection (its AllGather gates P2) ----
        wk_sb = wx.tile([P, NT, D], bf16)
        load_chunked(nc.sync, wk_sb, wkt)
        xk_sb = wx.tile([P, NT, S_SH], bf16)
        load_chunked(nc.sync, xk_sb, xkt)

        kt_sb_a = persist.tile([P, NT, KH], bf16)
        kt_sb_b = persist.tile([P, NT, KH], bf16)
        for et in range(NT):
            ps = psw.tile([P, FD], fp32, tag="work")
            for dt in range(NT):
                nc.tensor.matmul(
                    ps[:],
                    wk_sb[:, dt, et * P:(et + 1) * P],
                    xk_sb[:, dt, :],
                    start=(dt == 0),
                    stop=(dt == NT - 1),
                )
            nc.vector.tensor_copy(kt_sb_a[:, et, :], ps[:, 0:KH])
            nc.vector.tensor_copy(kt_sb_b[:, et, :], ps[:, KH:FD])
        nc.scalar.dma_start(kt_in_a[:], kt_sb_a[:])
        allgather(kt_in_a, ktg_a)
        nc.scalar.dma_start(kt_in_b[:], kt_sb_b[:])
        allgather(kt_in_b, ktg_b)

        # ---- P1b: Q^T projection (needed at P2 start) ----
        wq_sb = wx.tile([P, NT, D], bf16)
        load_chunked(nc.scalar, wq_sb, wqt)
        xq_sb = wx.tile([P, NT, S_SH], bf16)
        load_chunked(nc.scalar, xq_sb, xqt)

        qt_sb = persist.tile([P, NT, S_SH], bf16)
        for et in range(NT):
            ps = psw.tile([P, FD], fp32, tag="work")
            for dt in range(NT):
                nc.tensor.matmul(
                    ps[:],
                    wq_sb[:, dt, et * P:(et + 1) * P],
                    xq_sb[:, dt, :],
                    start=(dt == 0),
                    stop=(dt == NT - 1),
                )
            nc.vector.tensor_copy(qt_sb[:, et, :], ps[:])

        # ---- P1c: V projection (its AllGather gates P3) ----
        wv_sb = wx.tile([P, NT, D], bf16)
        load_chunked(nc.scalar, wv_sb, wvt)
        xv_sb = wx.tile([P, NT, S_SH], bf16)
        load_chunked(nc.scalar, xv_sb, xvt)

        v_sb_a = persist.tile([P, NQS, FD], bf16)
        v_sb_b = persist.tile([P, NQS, FD], bf16)
        for ec in range(NDC):
            tgt = v_sb_a if ec == 0 else v_sb_b
            for st in range(NQS):
                ps = psw.tile([P, FD], fp32, tag="work")
                for dt in range(NT):
                    nc.tensor.matmul(
                        ps[:],
                        xv_sb[:, dt, st * P:(st + 1) * P],
                        wv_sb[:, dt, ec * FD:(ec + 1) * FD],
                        start=(dt == 0),
                        stop=(dt == NT - 1),
                    )
                nc.vector.tensor_copy(tgt[:, st, :], ps[:])
            if ec == 0:
                nc.scalar.dma_start(v_in_a[:], v_sb_a[:])
                allgather(v_in_a, vg_a)
            else:
                nc.scalar.dma_start(v_in_b[:], v_sb_b[:])
                allgather(v_in_b, vg_b)

        # ---- P2: simsT = (Q K^T)^T tiles + exp ----
        exp_sb = persist.tile([P, NKT, S_SH], bf16)
        for b in range(N_CORES):
            ka = stage.tile([P, NT, KH], bf16, tag="ktga", name=f"ktga{b}")
            nc.sync.dma_start(ka[:], ktg_a[b])
            kb = stage.tile([P, NT, KH], bf16, tag="ktgb", name=f"ktgb{b}")
            nc.sync.dma_start(kb[:], ktg_b[b])
            for kw in range(NQS):
                half, kwi = (ka, kw) if kw < 2 else (kb, kw - 2)
                ps = psw.tile([P, FD], fp32, tag="work")
                for et in range(NT):
                    nc.tensor.matmul(
                        ps[:],
                        half[:, et, kwi * P:(kwi + 1) * P],
                        qt_sb[:, et, :],
                        start=(et == 0),
                        stop=(et == NT - 1),
                    )
                nc.scalar.activation(
                    exp_sb[:, b * NQS + kw, :],
                    ps[:],
                    mybir.ActivationFunctionType.Exp,
                    scale=SCALE,
                )

        # ---- P3: attn @ V with fused denominator ----
        ones_sb = persist.tile([P, 1], bf16)
        nc.vector.memset(ones_sb[:], 1.0)
        recips = [persist.tile([P, 1], fp32, name=f"recip{qs}") for qs in range(NQS)]
        stash = {}

        for dc in range(NDC):
            vg_src = vg_a if dc == 0 else vg_b
            accs = [
                psacc.tile([P, FD], fp32, tag="acc", name=f"acc{dc}_{qs}")
                for qs in range(NQS)
            ]
            den_qs = (0, 1) if dc == 0 else (2, 3)
            dens = {
                qs: psden.tile([P, 1], fp32, tag="den", name=f"den{qs}")
                for qs in den_qs
            }
            for b in range(N_CORES):
                vg_sb = stream.tile([P, NQS, FD], bf16, tag="vgtile", name=f"vg{dc}_{b}")
                nc.sync.dma_start(vg_sb[:], vg_src[b])
                for st in range(NQS):
                    kt = b * NQS + st
                    first, last = kt == 0, kt == NKT - 1
                    for qs in range(NQS):
                        lhsT = exp_sb[:, kt, qs * P:(qs + 1) * P]
                        nc.tensor.matmul(
                            accs[qs][:], lhsT, vg_sb[:, st, :], start=first, stop=last
                        )
                        if qs in dens:
                            nc.tensor.matmul(
                                dens[qs][:], lhsT, ones_sb[:], start=first, stop=last
                            )
            for qs in den_qs:
                den_sb = outp.tile([P, 1], fp32, tag="densb", name=f"densb{qs}")
                nc.vector.tensor_copy(den_sb[:], dens[qs][:])
                nc.vector.reciprocal(recips[qs][:], den_sb[:])
            if dc == 0:
                for qs in (0, 1):
                    o_sb = outp.tile([P, FD], fp32, tag="osb")
                    nc.vector.tensor_scalar_mul(o_sb[:], accs[qs][:], recips[qs][:])
                    nc.scalar.dma_start(out[qs * P:(qs + 1) * P, 0:FD], o_sb[:])
                for qs in (2, 3):
                    stash[qs] = persist.tile([P, FD], fp32, name=f"stash{qs}")
                    nc.vector.tensor_copy(stash[qs][:], accs[qs][:])
            else:
                for qs in range(NQS):
                    o_sb = outp.tile([P, FD], fp32, tag="osb")
                    nc.vector.tensor_scalar_mul(o_sb[:], accs[qs][:], recips[qs][:])
                    nc.scalar.dma_start(out[qs * P:(qs + 1) * P, FD:D], o_sb[:])
                for qs in (2, 3):
                    o_sb = outp.tile([P, FD], fp32, tag="osb")
                    nc.vector.tensor_scalar_mul(o_sb[:], stash[qs][:], recips[qs][:])
                    nc.scalar.dma_start(out[qs * P:(qs + 1) * P, 0:FD], o_sb[:])


def _build():
    import concourse.bacc as bacc
    import concourse.mybir as mybir
    import concourse.tile as tile

    bf16 = mybir.dt.bfloat16
    fp32 = mybir.dt.float32

    nc = bacc.Bacc("TRN2", target_bir_lowering=False, debug=False, num_devices=N_CORES)

    xqt = nc.dram_tensor("xqt", [P, NT, S_SH], bf16, kind="ExternalInput")
    xkt = nc.dram_tensor("xkt", [P, NT, S_SH], bf16, kind="ExternalInput")
    xvt = nc.dram_tensor("xvt", [P, NT, S_SH], bf16, kind="ExternalInput")
    wqt = nc.dram_tensor("wqt", [P, NT, D], bf16, kind="ExternalInput")
    wkt = nc.dram_tensor("wkt", [P, NT, D], bf16, kind="ExternalInput")
    wvt = nc.dram_tensor("wvt", [P, NT, D], bf16, kind="ExternalInput")
    out = nc.dram_tensor("out", [S_SH, D], fp32, kind="ExternalOutput")

    with tile.TileContext(nc) as tc:
        _body(tc, nc, mybir, xqt, xkt, xvt, wqt, wkt, wvt, out)
    nc.compile()
    return nc


def get_nc():
    if "nc" not in _CACHE:
        _CACHE["nc"] = _build()
    return _CACHE["nc"]


def _to_tiles_xT(x_shard):
    """[512, 1024] f32 -> x^T in SBUF tile layout [128, 8, 512] bf16."""
    bf = ml_dtypes.bfloat16
    return np.ascontiguousarray(
        x_shard.T.astype(bf).reshape(NT, P, S_SH).transpose(1, 0, 2)
    )


def _to_tiles_wT(w):
    """[1024, 1024] f32 -> W^T in SBUF tile layout [128, 8, 1024] bf16."""
    bf = ml_dtypes.bfloat16
    return np.ascontiguousarray(
        w.T.astype(bf).reshape(NT, P, D).transpose(1, 0, 2)
    )


def make_in_maps(encodings_for_q, encodings_for_k, encodings_for_v, W_q, W_k, W_v):
    wqt = _to_tiles_wT(W_q)
    wkt = _to_tiles_wT(W_k)
    wvt = _to_tiles_wT(W_v)
    in_maps = []
    for c in range(N_CORES):
        sl = slice(c * S_SH, (c + 1) * S_SH)
        in_maps.append({
            "xqt": _to_tiles_xT(encodings_for_q[sl]),
            "xkt": _to_tiles_xT(encodings_for_k[sl]),
            "xvt": _to_tiles_xT(encodings_for_v[sl]),
            "wqt": wqt,
            "wkt": wkt,
            "wvt": wvt,
        })
    return in_maps


def kernel(**inputs):
    from concourse.bass_utils import run_bass_kernel_spmd

    nc = get_nc()
    in_maps = make_in_maps(**inputs)
    res = run_bass_kernel_spmd(nc, in_maps, core_ids=list(range(N_CORES)))
    return np.concatenate(
        [np.asarray(res.results[c]["out"], dtype=np.float32) for c in range(N_CORES)],
        axis=0,
    )


# revision 7
# speedup vs baseline: 1.1609x; 1.1609x over previous
"""Distributed attention kernel for Trainium2 (8 NeuronCores).

Problem: out = softmax((x_q W_q^T)(x_k W_k^T)^T / sqrt(D)) (x_v W_v^T)
with SEQ=4096, D=1024, all f32.

Strategy (sequence parallel, sharded projections):
  - Shard all three encodings along the sequence dim: core c owns rows
    [c*512, (c+1)*512).
  - Host-side prep: transpose + cast to bf16 + permute into the exact
    SBUF tile layout [128, ...] so every DMA moves fat contiguous
    per-partition lines (4-16KB descriptors).
  - Each core computes Q^T (kept local), K^T and V for its shard, then
    AllGathers K^T and V across the 8 cores. Both gathers are split in
    two chunks (K^T along k, V along the output d-chunk) so downstream
    compute starts on the first half while the second is in flight.
  - simsT[k, q] = sum_e KT[e, k] * QT[e, q] computed tile-by-tile
    (output already transposed so attn@V needs no on-chip transpose of
    the softmax matrix).
  - exp via ScalarE with fused 1/32 scale. Logits are ~N(0,1) so the
    max-subtraction is unnecessary for f32/bf16 range.
  - attn@V in two passes over 512-wide d-chunks; the softmax denominator
    rides along as N=1 matmuls against a ones vector (same stationary
    operand, so the extra LDWEIGHTS pipelines away). Each PSUM
    accumulation chain owns a full bank (start=True zeroes the whole 2KB
    zero region), so the 4 denominator chains are split across the two
    d-passes: 4 acc banks + 2 den banks + 2 work banks = 8.
  - Row-normalize by 1/den, DMA out f32. The dc=0 results for q-subtiles
    2,3 are stashed unnormalized in SBUF until their denominators finish
    in the dc=1 pass.
"""

import numpy as np
import ml_dtypes

N_CORES = 8
SEQ = 4096
D = 1024
S_SH = SEQ // N_CORES  # 512 rows per core
P = 128
NT = D // P            # 8 tiles along d / e
NKT = SEQ // P         # 32 k-tiles
NQS = S_SH // P        # 4 q (and local-k) sub-tiles
FD = 512               # matmul free dim (one PSUM bank)
KH = S_SH // 2         # 256, k-half for the chunked K^T gather
NDC = D // FD          # 2 output d-chunks
SCALE = 1.0 / float(np.sqrt(D))

_CACHE = {}


def _body(tc, nc, mybir, xqt, xkt, xvt, wqt, wkt, wvt, out):
    bf16 = mybir.dt.bfloat16
    fp32 = mybir.dt.float32
    RG = [list(range(N_CORES))]

    def allgather(src, dst):
        nc.gpsimd.collective_compute(
            "AllGather",
            mybir.AluOpType.bypass,
            replica_groups=RG,
            ins=[src[:].opt()],
            outs=[dst[:].opt()],
        )

    def load_chunked(engine, sb, dram_t, nchunk=4):
        step = NT // nchunk
        for i in range(nchunk):
            engine.dma_start(sb[:, i * step:(i + 1) * step], dram_t[:, i * step:(i + 1) * step])

    with (
        tc.tile_pool(name="dram", bufs=1, space="DRAM") as dram,
        tc.tile_pool(name="wx", bufs=1) as wx,
        tc.tile_pool(name="persist", bufs=1) as persist,
        tc.tile_pool(name="stage", bufs=2) as stage,
        tc.tile_pool(name="stream", bufs=3) as stream,
        tc.tile_pool(name="outp", bufs=4) as outp,
        tc.tile_pool(name="psw", bufs=2, space="PSUM") as psw,
        tc.tile_pool(name="psacc", bufs=4, space="PSUM") as psacc,
        tc.tile_pool(name="psden", bufs=2, space="PSUM") as psden,
    ):
        kt_in = dram.tile([P, NT, S_SH], bf16)
        ktg = dram.tile([N_CORES, P, NT, S_SH], bf16, addr_space="Shared")
        v_in = dram.tile([P, NDC, NQS, FD], bf16)
        vg = dram.tile([N_CORES, P, NDC, NQS, FD], bf16, addr_space="Shared")

        # ---- P0: HAM warm-up burst so P1's matmuls run at 2.4 GHz ----
        dummy_sb = wx.tile([P, FD], bf16)
        nc.gpsimd.memset(dummy_sb[:], 0.0)
        ps_warm = psw.tile([P, FD], fp32, tag="work")
        N_WARM = 48
        for i in range(N_WARM):
            nc.tensor.matmul(
                ps_warm[:],
                dummy_sb[:, 0:P],
                dummy_sb[:],
                start=(i == 0),
                stop=(i == N_WARM - 1),
            )

        # ---- P1a: K^T projection (its AllGather gates P2) ----
        wk_sb = wx.tile([P, NT, D], bf16)
        load_chunked(nc.sync, wk_sb, wkt)
        xk_sb = wx.tile([P, NT, S_SH], bf16)
        load_chunked(nc.scalar, xk_sb, xkt)

        kt_sb = persist.tile([P, NT, S_SH], bf16)
        for et in range(NT):
            ps = psw.tile([P, FD], fp32, tag="work")
            for dt in range(NT):
                nc.tensor.matmul(
                    ps[:],
                    wk_sb[:, dt, et * P:(et + 1) * P],
                    xk_sb[:, dt, :],
                    start=(dt == 0),
                    stop=(dt == NT - 1),
                )
            nc.vector.tensor_copy(kt_sb[:, et, :], ps[:])
        nc.scalar.dma_start(kt_in[:], kt_sb[:])
        allgather(kt_in, ktg)

        # ---- P1b: V projection (its AllGather gates P3) ----
        wv_sb = wx.tile([P, NT, D], bf16)
        load_chunked(nc.sync, wv_sb, wvt)
        xv_sb = wx.tile([P, NT, S_SH], bf16)
        load_chunked(nc.scalar, xv_sb, xvt)

        v_sb = persist.tile([P, NDC, NQS, FD], bf16)
        for st in range(NQS):
            for ec in range(NDC):
                ps = psw.tile([P, FD], fp32, tag="work")
                for dt in range(NT):
                    nc.tensor.matmul(
                        ps[:],
                        xv_sb[:, dt, st * P:(st + 1) * P],
                        wv_sb[:, dt, ec * FD:(ec + 1) * FD],
                        start=(dt == 0),
                        stop=(dt == NT - 1),
                    )
                nc.vector.tensor_copy(v_sb[:, ec, st, :], ps[:])
        nc.scalar.dma_start(v_in[:], v_sb[:])
        allgather(v_in, vg)

        # ---- P1c: Q^T projection (needed at P2 start) ----
        wq_sb = wx.tile([P, NT, D], bf16)
        load_chunked(nc.sync, wq_sb, wqt)
        xq_sb = wx.tile([P, NT, S_SH], bf16)
        load_chunked(nc.scalar, xq_sb, xqt)

        qt_sb = persist.tile([P, NT, S_SH], bf16)
        for et in range(NT):
            ps = psw.tile([P, FD], fp32, tag="work")
            for dt in range(NT):
                nc.tensor.matmul(
                    ps[:],
                    wq_sb[:, dt, et * P:(et + 1) * P],
                    xq_sb[:, dt, :],
                    start=(dt == 0),
                    stop=(dt == NT - 1),
                )
            nc.vector.tensor_copy(qt_sb[:, et, :], ps[:])

        # ---- P2: simsT = (Q K^T)^T tiles + exp ----
        exp_sb = persist.tile([P, NKT, S_SH], bf16)
        for b in range(N_CORES):
            ktg_sb = stage.tile([P, NT, S_SH], bf16, tag="ktgblk", name=f"ktg{b}")
            nc.sync.dma_start(ktg_sb[:], ktg[b])
            for kw in range(NQS):
                ps = psw.tile([P, FD], fp32, tag="work")
                for et in range(NT):
                    nc.tensor.matmul(
                        ps[:],
                        ktg_sb[:, et, kw * P:(kw + 1) * P],
                        qt_sb[:, et, :],
                        start=(et == 0),
                        stop=(et == NT - 1),
                    )
                nc.scalar.activation(
                    exp_sb[:, b * NQS + kw, :],
                    ps[:],
                    mybir.ActivationFunctionType.Exp,
                    scale=SCALE,
                )

        # ---- P3: attn @ V with fused denominator ----
        ones_sb = persist.tile([P, 1], bf16)
        nc.vector.memset(ones_sb[:], 1.0)
        recips = [persist.tile([P, 1], fp32, name=f"recip{qs}") for qs in range(NQS)]
        stash = {}

        for dc in range(NDC):
            accs = [
                psacc.tile([P, FD], fp32, tag="acc", name=f"acc{dc}_{qs}")
                for qs in range(NQS)
            ]
            den_qs = (0, 1) if dc == 0 else (2, 3)
            dens = {
                qs: psden.tile([P, 1], fp32, tag="den", name=f"den{qs}")
                for qs in den_qs
            }
            for b in range(N_CORES):
                vg_sb = stream.tile([P, NQS, FD], bf16, tag="vgtile", name=f"vg{dc}_{b}")
                nc.sync.dma_start(vg_sb[:], vg[b, :, dc])
                for st in range(NQS):
                    kt = b * NQS + st
                    first, last = kt == 0, kt == NKT - 1
                    for qs in range(NQS):
                        lhsT = exp_sb[:, kt, qs * P:(qs + 1) * P]
                        nc.tensor.matmul(
                            accs[qs][:], lhsT, vg_sb[:, st, :], start=first, stop=last
                        )
                        if qs in dens:
                            nc.tensor.matmul(
                                dens[qs][:], lhsT, ones_sb[:], start=first, stop=last
                            )
            for qs in den_qs:
                den_sb = outp.tile([P, 1], fp32, tag="densb", name=f"densb{qs}")
                nc.vector.tensor_copy(den_sb[:], dens[qs][:])
                nc.vector.reciprocal(recips[qs][:], den_sb[:])
            if dc == 0:
                for qs in (0, 1):
                    o_sb = outp.tile([P, FD], fp32, tag="osb")
                    nc.vector.tensor_scalar_mul(o_sb[:], accs[qs][:], recips[qs][:])
                    nc.scalar.dma_start(out[qs * P:(qs + 1) * P, 0:FD], o_sb[:])
                for qs in (2, 3):
                    stash[qs] = persist.tile([P, FD], fp32, name=f"stash{qs}")
                    nc.vector.tensor_copy(stash[qs][:], accs[qs][:])
            else:
                for qs in range(NQS):
                    o_sb = outp.tile([P, FD], fp32, tag="osb")
                    nc.vector.tensor_scalar_mul(o_sb[:], accs[qs][:], recips[qs][:])
                    nc.scalar.dma_start(out[qs * P:(qs + 1) * P, FD:D], o_sb[:])
                for qs in (2, 3):
                    o_sb = outp.tile([P, FD], fp32, tag="osb")
                    nc.vector.tensor_scalar_mul(o_sb[:], stash[qs][:], recips[qs][:])
                    nc.scalar.dma_start(out[qs * P:(qs + 1) * P, 0:FD], o_sb[:])


def _build():
    import concourse.bacc as bacc
    import concourse.mybir as mybir
    import concourse.tile as tile

    bf16 = mybir.dt.bfloat16
    fp32 = mybir.dt.float32

    nc = bacc.Bacc("TRN2", target_bir_lowering=False, debug=False, num_devices=N_CORES)

    xqt = nc.dram_tensor("xqt", [P, NT, S_SH], bf16, kind="ExternalInput")
    xkt = nc.dram_tensor("xkt", [P, NT, S_SH], bf16, kind="ExternalInput")
    xvt = nc.dram_tensor("xvt", [P, NT, S_SH], bf16, kind="ExternalInput")
    wqt = nc.dram_tensor("wqt", [P, NT, D], bf16, kind="ExternalInput")
    wkt = nc.dram_tensor("wkt", [P, NT, D], bf16, kind="ExternalInput")
    wvt = nc.dram_tensor("wvt", [P, NT, D], bf16, kind="ExternalInput")
    out = nc.dram_tensor("out", [S_SH, D], fp32, kind="ExternalOutput")

    with tile.TileContext(nc) as tc:
        _body(tc, nc, mybir, xqt, xkt, xvt, wqt, wkt, wvt, out)
    nc.compile()
    return nc


def get_nc():
    if "nc" not in _CACHE:
        _CACHE["nc"] = _build()
    return _CACHE["nc"]


def _to_tiles_xT(x_shard):
    """[512, 1024] f32 -> x^T in SBUF tile layout [128, 8, 512] bf16."""
    bf = ml_dtypes.bfloat16
    return np.ascontiguousarray(
        x_shard.T.astype(bf).reshape(NT, P, S_SH).transpose(1, 0, 2)
    )


def _to_tiles_wT(w):
    """[1024, 1024] f32 -> W^T in SBUF tile layout [128, 8, 1024] bf16."""
    bf = ml_dtypes.bfloat16
    return np.ascontiguousarray(
        w.T.astype(bf).reshape(NT, P, D).transpose(1, 0, 2)
    )


def make_in_maps(encodings_for_q, encodings_for_k, encodings_for_v, W_q, W_k, W_v):
    wqt = _to_tiles_wT(W_q)
    wkt = _to_tiles_wT(W_k)
    wvt = _to_tiles_wT(W_v)
    in_maps = []
    for c in range(N_CORES):
        sl = slice(c * S_SH, (c + 1) * S_SH)
        in_maps.append({
            "xqt": _to_tiles_xT(encodings_for_q[sl]),
            "xkt": _to_tiles_xT(encodings_for_k[sl]),
            "xvt": _to_tiles_xT(encodings_for_v[sl]),
            "wqt": wqt,
            "wkt": wkt,
            "wvt": wvt,
        })
    return in_maps


def kernel(**inputs):
    from concourse.bass_utils import run_bass_kernel_spmd

    nc = get_nc()
    in_maps = make_in_maps(**inputs)
    res = run_bass_kernel_spmd(nc, in_maps, core_ids=list(range(N_CORES)))
    return np.concatenate(
        [np.asarray(res.results[c]["out"], dtype=np.float32) for c in range(N_CORES)],
        axis=0,
    )


# revision 10
# speedup vs baseline: 1.1771x; 1.0139x over previous
"""Distributed attention kernel for Trainium2 (8 NeuronCores).

Problem: out = softmax((x_q W_q^T)(x_k W_k^T)^T / sqrt(D)) (x_v W_v^T)
with SEQ=4096, D=1024, all f32.

Strategy (sequence parallel, sharded projections):
  - Shard all three encodings along the sequence dim: core c owns rows
    [c*512, (c+1)*512).
  - Host-side prep: transpose + cast to bf16 + permute into the exact
    SBUF tile layout [128, ...] so every DMA moves fat contiguous
    per-partition lines (4-16KB descriptors).
  - Each core computes Q^T (kept local), K^T and V for its shard, then
    AllGathers K^T and V across the 8 cores. Both gathers are split in
    two chunks (K^T along k, V along the output d-chunk) so downstream
    compute starts on the first half while the second is in flight.
  - simsT[k, q] = sum_e KT[e, k] * QT[e, q] computed tile-by-tile
    (output already transposed so attn@V needs no on-chip transpose of
    the softmax matrix).
  - exp via ScalarE with fused 1/32 scale. Logits are ~N(0,1) so the
    max-subtraction is unnecessary for f32/bf16 range.
  - attn@V in two passes over 512-wide d-chunks; the softmax denominator
    rides along as N=1 matmuls against a ones vector (same stationary
    operand, so the extra LDWEIGHTS pipelines away). Each PSUM
    accumulation chain owns a full bank (start=True zeroes the whole 2KB
    zero region), so the 4 denominator chains are split across the two
    d-passes: 4 acc banks + 2 den banks + 2 work banks = 8.
  - Row-normalize by 1/den, DMA out f32. The dc=0 results for q-subtiles
    2,3 are stashed unnormalized in SBUF until their denominators finish
    in the dc=1 pass.
"""

import numpy as np
import ml_dtypes

N_CORES = 8
SEQ = 4096
D = 1024
S_SH = SEQ // N_CORES  # 512 rows per core
P = 128
NT = D // P            # 8 tiles along d / e
NKT = SEQ // P         # 32 k-tiles
NQS = S_SH // P        # 4 q (and local-k) sub-tiles
FD = 512               # matmul free dim (one PSUM bank)
KH = S_SH // 2         # 256, k-half for the chunked K^T gather
NDC = D // FD          # 2 output d-chunks
SCALE = 1.0 / float(np.sqrt(D))

_CACHE = {}


def _body(tc, nc, mybir, xqt, xkt, xvt, wqt, wkt, wvt, out):
    bf16 = mybir.dt.bfloat16
    fp32 = mybir.dt.float32
    RG = [list(range(N_CORES))]

    def allgather(src, dst):
        nc.gpsimd.collective_compute(
            "AllGather",
            mybir.AluOpType.bypass,
            replica_groups=RG,
            ins=[src[:].opt()],
            outs=[dst[:].opt()],
        )

    def load_chunked(engine, sb, dram_t, nchunk=4):
        step = NT // nchunk
        for i in range(nchunk):
            engine.dma_start(sb[:, i * step:(i + 1) * step], dram_t[:, i * step:(i + 1) * step])

    with (
        tc.tile_pool(name="dram", bufs=1, space="DRAM") as dram,
        tc.tile_pool(name="wx", bufs=1) as wx,
        tc.tile_pool(name="persist", bufs=1) as persist,
        tc.tile_pool(name="stage", bufs=2) as stage,
        tc.tile_pool(name="stream", bufs=3) as stream,
        tc.tile_pool(name="outp", bufs=4) as outp,
        tc.tile_pool(name="psw", bufs=2, space="PSUM") as psw,
        tc.tile_pool(name="psacc", bufs=4, space="PSUM") as psacc,
        tc.tile_pool(name="psden", bufs=2, space="PSUM") as psden,
    ):
        kt_in = dram.tile([P, NT, S_SH], bf16)
        ktg = dram.tile([N_CORES, P, NT, S_SH], bf16, addr_space="Shared")
        v_in = dram.tile([P, NDC, NQS, FD], bf16)
        vg = dram.tile([N_CORES, P, NDC, NQS, FD], bf16, addr_space="Shared")

        # ---- P0: HAM warm-up burst so P1's matmuls run at 2.4 GHz ----
        dummy_sb = wx.tile([P, FD], bf16)
        nc.gpsimd.memset(dummy_sb[:], 0.0)
        ps_warm = psw.tile([P, FD], fp32, tag="work")
        N_WARM = 36
        for i in range(N_WARM):
            nc.tensor.matmul(
                ps_warm[:],
                dummy_sb[:, 0:P],
                dummy_sb[:],
                start=(i == 0),
                stop=(i == N_WARM - 1),
            )

        # ---- P1a: K^T projection (its AllGather gates P2) ----
        wk_sb = wx.tile([P, NT, D], bf16)
        load_chunked(nc.sync, wk_sb, wkt)
        xk_sb = wx.tile([P, NT, S_SH], bf16)
        load_chunked(nc.scalar, xk_sb, xkt)

        kt_sb = persist.tile([P, NT, S_SH], bf16)
        for et in range(NT):
            ps = psw.tile([P, FD], fp32, tag="work")
            for dt in range(NT):
                nc.tensor.matmul(
                    ps[:],
                    wk_sb[:, dt, et * P:(et + 1) * P],
                    xk_sb[:, dt, :],
                    start=(dt == 0),
                    stop=(dt == NT - 1),
                )
            nc.vector.tensor_copy(kt_sb[:, et, :], ps[:])
            nc.scalar.dma_start(kt_in[:, et], kt_sb[:, et])
        allgather(kt_in, ktg)

        # ---- P1b: V projection (its AllGather gates P3) ----
        wv_sb = wx.tile([P, NT, D], bf16)
        load_chunked(nc.sync, wv_sb, wvt)
        xv_sb = wx.tile([P, NT, S_SH], bf16)
        load_chunked(nc.scalar, xv_sb, xvt)

        v_sb = persist.tile([P, NDC, NQS, FD], bf16)
        for st in range(NQS):
            for ec in range(NDC):
                ps = psw.tile([P, FD], fp32, tag="work")
                for dt in range(NT):
                    nc.tensor.matmul(
                        ps[:],
                        xv_sb[:, dt, st * P:(st + 1) * P],
                        wv_sb[:, dt, ec * FD:(ec + 1) * FD],
                        start=(dt == 0),
                        stop=(dt == NT - 1),
                    )
                nc.vector.tensor_copy(v_sb[:, ec, st, :], ps[:])
                nc.scalar.dma_start(v_in[:, ec, st], v_sb[:, ec, st])
        allgather(v_in, vg)

        # ---- P1c: Q^T projection (needed at P2 start) ----
        wq_sb = wx.tile([P, NT, D], bf16)
        load_chunked(nc.sync, wq_sb, wqt)
        xq_sb = wx.tile([P, NT, S_SH], bf16)
        load_chunked(nc.scalar, xq_sb, xqt)

        qt_sb = persist.tile([P, NT, S_SH], bf16)
        for et in range(NT):
            ps = psw.tile([P, FD], fp32, tag="work")
            for dt in range(NT):
                nc.tensor.matmul(
                    ps[:],
                    wq_sb[:, dt, et * P:(et + 1) * P],
                    xq_sb[:, dt, :],
                    start=(dt == 0),
                    stop=(dt == NT - 1),
                )
            nc.vector.tensor_copy(qt_sb[:, et, :], ps[:])

        # ---- P2: simsT = (Q K^T)^T tiles + exp ----
        exp_sb = persist.tile([P, NKT, S_SH], bf16)
        for b in range(N_CORES):
            ktg_sb = stage.tile([P, NT, S_SH], bf16, tag="ktgblk", name=f"ktg{b}")
            nc.sync.dma_start(ktg_sb[:], ktg[b])
            for kw in range(NQS):
                ps = psw.tile([P, FD], fp32, tag="work")
                for et in range(NT):
                    nc.tensor.matmul(
                        ps[:],
                        ktg_sb[:, et, kw * P:(kw + 1) * P],
                        qt_sb[:, et, :],
                        start=(et == 0),
                        stop=(et == NT - 1),
                    )
                nc.scalar.activation(
                    exp_sb[:, b * NQS + kw, :],
                    ps[:],
                    mybir.ActivationFunctionType.Exp,
                    scale=SCALE,
                )

        # ---- P3: attn @ V with fused denominator ----
        ones_sb = persist.tile([P, 1], bf16)
        nc.vector.memset(ones_sb[:], 1.0)
        recips = [persist.tile([P, 1], fp32, name=f"recip{qs}") for qs in range(NQS)]
        stash = {}

        for dc in range(NDC):
            accs = [
                psacc.tile([P, FD], fp32, tag="acc", name=f"acc{dc}_{qs}")
                for qs in range(NQS)
            ]
            den_qs = (0, 1) if dc == 0 else (2, 3)
            dens = {
                qs: psden.tile([P, 1], fp32, tag="den", name=f"den{qs}")
                for qs in den_qs
            }
            for b in range(N_CORES):
                vg_sb = stream.tile([P, NQS, FD], bf16, tag="vgtile", name=f"vg{dc}_{b}")
                nc.sync.dma_start(vg_sb[:], vg[b, :, dc])
                for st in range(NQS):
                    kt = b * NQS + st
                    first, last = kt == 0, kt == NKT - 1
                    for qs in range(NQS):
                        lhsT = exp_sb[:, kt, qs * P:(qs + 1) * P]
                        nc.tensor.matmul(
                            accs[qs][:], lhsT, vg_sb[:, st, :], start=first, stop=last
                        )
                        if qs in dens:
                            nc.tensor.matmul(
                                dens[qs][:], lhsT, ones_sb[:], start=first, stop=last
                            )
            for qs in den_qs:
                den_sb = outp.tile([P, 1], fp32, tag="densb", name=f"densb{qs}")
                nc.vector.tensor_copy(den_sb[:], dens[qs][:])
                nc.vector.reciprocal(recips[qs][:], den_sb[:])
            if dc == 0:
                for qs in (0, 1):
                    o_sb = outp.tile([P, FD], fp32, tag="osb")
                    nc.vector.tensor_scalar_mul(o_sb[:], accs[qs][:], recips[qs][:])
                    nc.scalar.dma_start(out[qs * P:(qs + 1) * P, 0:FD], o_sb[:])
                for qs in (2, 3):
                    stash[qs] = persist.tile([P, FD], fp32, name=f"stash{qs}")
                    nc.vector.tensor_copy(stash[qs][:], accs[qs][:])
            else:
                for qs in range(NQS):
                    o_sb = outp.tile([P, FD], fp32, tag="osb")
                    nc.vector.tensor_scalar_mul(o_sb[:], accs[qs][:], recips[qs][:])
                    nc.scalar.dma_start(out[qs * P:(qs + 1) * P, FD:D], o_sb[:])
                for qs in (2, 3):
                    o_sb = outp.tile([P, FD], fp32, tag="osb")
                    nc.vector.tensor_scalar_mul(o_sb[:], stash[qs][:], recips[qs][:])
                    nc.scalar.dma_start(out[qs * P:(qs + 1) * P, 0:FD], o_sb[:])


def _build():
    import concourse.bacc as bacc
    import concourse.mybir as mybir
    import concourse.tile as tile

    bf16 = mybir.dt.bfloat16
    fp32 = mybir.dt.float32

    nc = bacc.Bacc("TRN2", target_bir_lowering=False, debug=False, num_devices=N_CORES)

    xqt = nc.dram_tensor("xqt", [P, NT, S_SH], bf16, kind="ExternalInput")
    xkt = nc.dram_tensor("xkt", [P, NT, S_SH], bf16, kind="ExternalInput")
    xvt = nc.dram_tensor("xvt", [P, NT, S_SH], bf16, kind="ExternalInput")
    wqt = nc.dram_tensor("wqt", [P, NT, D], bf16, kind="ExternalInput")
    wkt = nc.dram_tensor("wkt", [P, NT, D], bf16, kind="ExternalInput")
    wvt = nc.dram_tensor("wvt", [P, NT, D], bf16, kind="ExternalInput")
    out = nc.dram_tensor("out", [S_SH, D], fp32, kind="ExternalOutput")

    with tile.TileContext(nc) as tc:
        _body(tc, nc, mybir, xqt, xkt, xvt, wqt, wkt, wvt, out)
    nc.compile()
    return nc


def get_nc():
    if "nc" not in _CACHE:
        _CACHE["nc"] = _build()
    return _CACHE["nc"]


def _to_tiles_xT(x_shard):
    """[512, 1024] f32 -> x^T in SBUF tile layout [128, 8, 512] bf16."""
    bf = ml_dtypes.bfloat16
    return np.ascontiguousarray(
        x_shard.T.astype(bf).reshape(NT, P, S_SH).transpose(1, 0, 2)
    )


def _to_tiles_wT(w):
    """[1024, 1024] f32 -> W^T in SBUF tile layout [128, 8, 1024] bf16."""
    bf = ml_dtypes.bfloat16
    return np.ascontiguousarray(
        w.T.astype(bf).reshape(NT, P, D).transpose(1, 0, 2)
    )


def make_in_maps(encodings_for_q, encodings_for_k, encodings_for_v, W_q, W_k, W_v):
    wqt = _to_tiles_wT(W_q)
    wkt = _to_tiles_wT(W_k)
    wvt = _to_tiles_wT(W_v)
    in_maps = []
    for c in range(N_CORES):
        sl = slice(c * S_SH, (c + 1) * S_SH)
        in_maps.append({
            "xqt": _to_tiles_xT(encodings_for_q[sl]),
            "xkt": _to_tiles_xT(encodings_for_k[sl]),
            "xvt": _to_tiles_xT(encodings_for_v[sl]),
            "wqt": wqt,
            "wkt": wkt,
            "wvt": wvt,
        })
    return in_maps


def kernel(**inputs):
    from concourse.bass_utils import run_bass_kernel_spmd

    nc = get_nc()
    in_maps = make_in_maps(**inputs)
    res = run_bass_kernel_spmd(nc, in_maps, core_ids=list(range(N_CORES)))
    return np.concatenate(
        [np.asarray(res.results[c]["out"], dtype=np.float32) for c in range(N_CORES)],
        axis=0,
    )


# revision 12
# speedup vs baseline: 1.2353x; 1.0494x over previous
"""Distributed attention kernel for Trainium2 (8 NeuronCores).

Problem: out = softmax((x_q W_q^T)(x_k W_k^T)^T / sqrt(D)) (x_v W_v^T)
with SEQ=4096, D=1024, all f32.

Strategy (sequence parallel, sharded projections):
  - Shard all three encodings along the sequence dim: core c owns rows
    [c*512, (c+1)*512).
  - Host-side prep: transpose + cast to bf16 + permute into the exact
    SBUF tile layout [128, ...] so every DMA moves fat contiguous
    per-partition lines (4-16KB descriptors).
  - Each core computes Q^T (kept local), K^T and V for its shard, then
    AllGathers K^T and V across the 8 cores. Both gathers are split in
    two chunks (K^T along k, V along the output d-chunk) so downstream
    compute starts on the first half while the second is in flight.
  - simsT[k, q] = sum_e KT[e, k] * QT[e, q] computed tile-by-tile
    (output already transposed so attn@V needs no on-chip transpose of
    the softmax matrix).
  - exp via ScalarE with fused 1/32 scale. Logits are ~N(0,1) so the
    max-subtraction is unnecessary for f32/bf16 range.
  - attn@V in two passes over 512-wide d-chunks; the softmax denominator
    rides along as N=1 matmuls against a ones vector (same stationary
    operand, so the extra LDWEIGHTS pipelines away). Each PSUM
    accumulation chain owns a full bank (start=True zeroes the whole 2KB
    zero region), so the 4 denominator chains are split across the two
    d-passes: 4 acc banks + 2 den banks + 2 work banks = 8.
  - Row-normalize by 1/den, DMA out f32. The dc=0 results for q-subtiles
    2,3 are stashed unnormalized in SBUF until their denominators finish
    in the dc=1 pass.
"""

import numpy as np
import ml_dtypes

N_CORES = 8
SEQ = 4096
D = 1024
S_SH = SEQ // N_CORES  # 512 rows per core
P = 128
NT = D // P            # 8 tiles along d / e
NKT = SEQ // P         # 32 k-tiles
NQS = S_SH // P        # 4 q (and local-k) sub-tiles
FD = 512               # matmul free dim (one PSUM bank)
KH = S_SH // 2         # 256, k-half for the chunked K^T gather
NDC = D // FD          # 2 output d-chunks
SCALE = 1.0 / float(np.sqrt(D))

_CACHE = {}


def _body(tc, nc, mybir, xqt, xkt, xvt, wqt, wkt, wvt, out):
    bf16 = mybir.dt.bfloat16
    fp32 = mybir.dt.float32
    RG = [list(range(N_CORES))]

    def allgather(src, dst):
        nc.gpsimd.collective_compute(
            "AllGather",
            mybir.AluOpType.bypass,
            replica_groups=RG,
            ins=[src[:].opt()],
            outs=[dst[:].opt()],
        )

    def load_chunked(_engine, sb, dram_t, nchunk=4):
        step = NT // nchunk
        for i in range(nchunk):
            eng = nc.sync if i % 2 == 0 else nc.scalar
            eng.dma_start(sb[:, i * step:(i + 1) * step], dram_t[:, i * step:(i + 1) * step])

    with (
        tc.tile_pool(name="dram", bufs=1, space="DRAM") as dram,
        tc.tile_pool(name="wx", bufs=1) as wx,
        tc.tile_pool(name="persist", bufs=1) as persist,
        tc.tile_pool(name="stage", bufs=3) as stage,
        tc.tile_pool(name="stream", bufs=3) as stream,
        tc.tile_pool(name="outp", bufs=4) as outp,
        tc.tile_pool(name="psw", bufs=2, space="PSUM") as psw,
        tc.tile_pool(name="psacc", bufs=4, space="PSUM") as psacc,
        tc.tile_pool(name="psden", bufs=2, space="PSUM") as psden,
    ):
        kt_in = dram.tile([P, NT, S_SH], bf16)
        ktg = dram.tile([N_CORES, P, NT, S_SH], bf16, addr_space="Shared")
        v_in = dram.tile([P, NDC, NQS, FD], bf16)
        vg = dram.tile([N_CORES, P, NDC, NQS, FD], bf16, addr_space="Shared")

        # ---- P0: HAM warm-up burst so P1's matmuls run at 2.4 GHz ----
        dummy_sb = wx.tile([P, FD], bf16)
        nc.gpsimd.memset(dummy_sb[:], 0.0)
        ps_warm = psw.tile([P, FD], fp32, tag="work")
        N_WARM = 36
        for i in range(N_WARM):
            nc.tensor.matmul(
                ps_warm[:],
                dummy_sb[:, 0:P],
                dummy_sb[:],
                start=(i == 0),
                stop=(i == N_WARM - 1),
            )

        # ---- P1a: K^T projection (its AllGather gates P2) ----
        wk_sb = wx.tile([P, NT, D], bf16)
        load_chunked(nc.sync, wk_sb, wkt)
        xk_sb = wx.tile([P, NT, S_SH], bf16)
        load_chunked(nc.scalar, xk_sb, xkt)

        kt_sb = persist.tile([P, NT, S_SH], bf16)
        for et in range(NT):
            ps = psw.tile([P, FD], fp32, tag="work")
            for dt in range(NT):
                nc.tensor.matmul(
                    ps[:],
                    wk_sb[:, dt, et * P:(et + 1) * P],
                    xk_sb[:, dt, :],
                    start=(dt == 0),
                    stop=(dt == NT - 1),
                )
            nc.vector.tensor_copy(kt_sb[:, et, :], ps[:])
            nc.scalar.dma_start(kt_in[:, et], kt_sb[:, et])
        allgather(kt_in, ktg)

        # ---- P1b: V projection (its AllGather gates P3) ----
        wv_sb = wx.tile([P, NT, D], bf16)
        load_chunked(nc.sync, wv_sb, wvt)
        xv_sb = wx.tile([P, NT, S_SH], bf16)
        load_chunked(nc.scalar, xv_sb, xvt)

        v_sb = persist.tile([P, NDC, NQS, FD], bf16)
        for st in range(NQS):
            for ec in range(NDC):
                ps = psw.tile([P, FD], fp32, tag="work")
                for dt in range(NT):
                    nc.tensor.matmul(
                        ps[:],
                        xv_sb[:, dt, st * P:(st + 1) * P],
                        wv_sb[:, dt, ec * FD:(ec + 1) * FD],
                        start=(dt == 0),
                        stop=(dt == NT - 1),
                    )
                nc.vector.tensor_copy(v_sb[:, ec, st, :], ps[:])
                nc.scalar.dma_start(v_in[:, ec, st], v_sb[:, ec, st])
        allgather(v_in, vg)

        # ---- P1c: Q^T projection (needed at P2 start) ----
        wq_sb = wx.tile([P, NT, D], bf16)
        load_chunked(nc.sync, wq_sb, wqt)
        xq_sb = wx.tile([P, NT, S_SH], bf16)
        load_chunked(nc.scalar, xq_sb, xqt)

        qt_sb = persist.tile([P, NT, S_SH], bf16)
        for et in range(NT):
            ps = psw.tile([P, FD], fp32, tag="work")
            for dt in range(NT):
                nc.tensor.matmul(
                    ps[:],
                    wq_sb[:, dt, et * P:(et + 1) * P],
                    xq_sb[:, dt, :],
                    start=(dt == 0),
                    stop=(dt == NT - 1),
                )
            nc.vector.tensor_copy(qt_sb[:, et, :], ps[:])

        # ---- P2: simsT = (Q K^T)^T tiles + exp ----
        exp_sb = persist.tile([P, NKT, S_SH], bf16)
        for b in range(N_CORES):
            ktg_sb = stage.tile([P, NT, S_SH], bf16, tag="ktgblk", name=f"ktg{b}")
            nc.sync.dma_start(ktg_sb[:], ktg[b])
            for kw in range(NQS):
                ps = psw.tile([P, FD], fp32, tag="work")
                for et in range(NT):
                    nc.tensor.matmul(
                        ps[:],
                        ktg_sb[:, et, kw * P:(kw + 1) * P],
                        qt_sb[:, et, :],
                        start=(et == 0),
                        stop=(et == NT - 1),
                    )
                nc.scalar.activation(
                    exp_sb[:, b * NQS + kw, :],
                    ps[:],
                    mybir.ActivationFunctionType.Exp,
                    scale=SCALE,
                )

        # ---- P3: attn @ V with fused denominator ----
        ones_sb = persist.tile([P, 1], bf16)
        nc.vector.memset(ones_sb[:], 1.0)
        recips = [persist.tile([P, 1], fp32, name=f"recip{qs}") for qs in range(NQS)]
        stash = {}

        for dc in range(NDC):
            accs = [
                psacc.tile([P, FD], fp32, tag="acc", name=f"acc{dc}_{qs}")
                for qs in range(NQS)
            ]
            den_qs = (0, 1) if dc == 0 else (2, 3)
            dens = {
                qs: psden.tile([P, 1], fp32, tag="den", name=f"den{qs}")
                for qs in den_qs
            }
            for b in range(N_CORES):
                vg_sb = stream.tile([P, NQS, FD], bf16, tag="vgtile", name=f"vg{dc}_{b}")
                nc.sync.dma_start(vg_sb[:], vg[b, :, dc])
                for st in range(NQS):
                    kt = b * NQS + st
                    first, last = kt == 0, kt == NKT - 1
                    for qs in range(NQS):
                        lhsT = exp_sb[:, kt, qs * P:(qs + 1) * P]
                        nc.tensor.matmul(
                            accs[qs][:], lhsT, vg_sb[:, st, :], start=first, stop=last
                        )
                        if qs in dens:
                            nc.tensor.matmul(
                                dens[qs][:], lhsT, ones_sb[:], start=first, stop=last
                            )
            for qs in den_qs:
                den_sb = outp.tile([P, 1], fp32, tag="densb", name=f"densb{qs}")
                nc.vector.tensor_copy(den_sb[:], dens[qs][:])
                nc.vector.reciprocal(recips[qs][:], den_sb[:])
            if dc == 0:
                for qs in (0, 1):
                    o_sb = outp.tile([P, FD], fp32, tag="osb")
                    nc.vector.tensor_scalar_mul(o_sb[:], accs[qs][:], recips[qs][:])
                    nc.scalar.dma_start(out[qs * P:(qs + 1) * P, 0:FD], o_sb[:])
                for qs in (2, 3):
                    stash[qs] = persist.tile([P, FD], fp32, name=f"stash{qs}")
                    nc.vector.tensor_copy(stash[qs][:], accs[qs][:])
            else:
                for qs in range(NQS):
                    o_sb = outp.tile([P, FD], fp32, tag="osb")
                    nc.vector.tensor_scalar_mul(o_sb[:], accs[qs][:], recips[qs][:])
                    nc.scalar.dma_start(out[qs * P:(qs + 1) * P, FD:D], o_sb[:])
                for qs in (2, 3):
                    o_sb = outp.tile([P, FD], fp32, tag="osb")
                    nc.vector.tensor_scalar_mul(o_sb[:], stash[qs][:], recips[qs][:])
                    nc.scalar.dma_start(out[qs * P:(qs + 1) * P, 0:FD], o_sb[:])


def _build():
    import concourse.bacc as bacc
    import concourse.mybir as mybir
    import concourse.tile as tile

    bf16 = mybir.dt.bfloat16
    fp32 = mybir.dt.float32

    nc = bacc.Bacc("TRN2", target_bir_lowering=False, debug=False, num_devices=N_CORES)

    xqt = nc.dram_tensor("xqt", [P, NT, S_SH], bf16, kind="ExternalInput")
    xkt = nc.dram_tensor("xkt", [P, NT, S_SH], bf16, kind="ExternalInput")
    xvt = nc.dram_tensor("xvt", [P, NT, S_SH], bf16, kind="ExternalInput")
    wqt = nc.dram_tensor("wqt", [P, NT, D], bf16, kind="ExternalInput")
    wkt = nc.dram_tensor("wkt", [P, NT, D], bf16, kind="ExternalInput")
    wvt = nc.dram_tensor("wvt", [P, NT, D], bf16, kind="ExternalInput")
    out = nc.dram_tensor("out", [S_SH, D], fp32, kind="ExternalOutput")

    with tile.TileContext(nc) as tc:
        _body(tc, nc, mybir, xqt, xkt, xvt, wqt, wkt, wvt, out)
    nc.compile()
    return nc


def get_nc():
    if "nc" not in _CACHE:
        _CACHE["nc"] = _build()
    return _CACHE["nc"]


def _to_tiles_xT(x_shard):
    """[512, 1024] f32 -> x^T in SBUF tile layout [128, 8, 512] bf16."""
    bf = ml_dtypes.bfloat16
    return np.ascontiguousarray(
        x_shard.T.astype(bf).reshape(NT, P, S_SH).transpose(1, 0, 2)
    )


def _to_tiles_wT(w):
    """[1024, 1024] f32 -> W^T in SBUF tile layout [128, 8, 1024] bf16."""
    bf = ml_dtypes.bfloat16
    return np.ascontiguousarray(
        w.T.astype(bf).reshape(NT, P, D).transpose(1, 0, 2)
    )


def make_in_maps(encodings_for_q, encodings_for_k, encodings_for_v, W_q, W_k, W_v):
    wqt = _to_tiles_wT(W_q)
    wkt = _to_tiles_wT(W_k)
    wvt = _to_tiles_wT(W_v)
    in_maps = []
    for c in range(N_CORES):
        sl = slice(c * S_SH, (c + 1) * S_SH)
        in_maps.append({
            "xqt": _to_tiles_xT(encodings_for_q[sl]),
            "xkt": _to_tiles_xT(encodings_for_k[sl]),
            "xvt": _to_tiles_xT(encodings_for_v[sl]),
            "wqt": wqt,
            "wkt": wkt,
            "wvt": wvt,
        })
    return in_maps


def kernel(**inputs):
    from concourse.bass_utils import run_bass_kernel_spmd

    nc = get_nc()
    in_maps = make_in_maps(**inputs)
    res = run_bass_kernel_spmd(nc, in_maps, core_ids=list(range(N_CORES)))
    return np.concatenate(
        [np.asarray(res.results[c]["out"], dtype=np.float32) for c in range(N_CORES)],
        axis=0,
    )
